# revision 1
# baseline (speedup 1.0000x reference)
"""TRN2 Bass kernel for nn_OFTLinear (forward).

Math: the whole OFT chain is linear, so
    out = x @ W_eff + b_eff
with
    W_eff = P_in . BD(R_right) . W^T . BD(R_left) . P_out      [2048 x 2048]
    b_eff = (BD(R_left)^T b)[inv_perm_out]
where R = Cayley-Neumann(skew(oft)) per 32x32 block, BD() is block-diagonal,
and P_in/P_out are the input/output feature permutations.

Device pipeline (replicated on all 8 cores; x sharded along tokens):
  Q:  Q_flat = vec^T @ E (E: host-built one-hot skew-scatter matrix)
  C:  BD4 tiles of Q (4 blocks per 128x128 tile) -> Cayley powers on PE ->
      R_left tiles (g<16) and R_right^T = R(-Q) tiles (g>=16)
  H:  H = BD(R_left)^T @ W on PE, plain-stored to DRAM [out, in]
  T:  h2row_g = row-gather of H by inv_perm_out (dma_gather, 4 SWDGE queues)
      -> PE-transpose into H2T tiles [in, out]
  G:  G2_i = BD(R_right) @ H2T_i on PE, plain-stored [in, out] (float32r)
  GEMM: W_eff k-tiles = row-gather of G2 by inv_perm_in; out = xT.T@W_eff + b
      (float32r matmuls, fp32 accumulate)

Host does layout-only work: shard x along tokens, transpose each shard
(fp32 DMA transpose is unsupported on this stack), concat oft_L/oft_R, and
build integer index/one-hot constants from the permutation/index buffers.
"""

import numpy as np

IN_F = 2048
OUT_F = 2048
BS = 32
N_ELEM = BS * (BS - 1) // 2  # 496
N_BLOCKS = 128  # 64 left + 64 right
N_CORES = 8
TOKENS = 4 * 8192
TOKPC = TOKENS // N_CORES  # 4096
KB = IN_F // 128  # 16 k-blocks
NB = OUT_F // 128  # 16 n-blocks

_CACHE = {}


def _build(tokpc, use_f32r=True):
    import os
    qmode = os.environ.get("GATHER_QMODE", "q0")
    if qmode == "q0":
        qsel = lambda j: 0
    elif qmode == "rr3":
        qsel = lambda j: 1 + (j % 3)
    else:
        qsel = lambda j: j % 4
    import concourse.bass as bass
    import concourse.bacc as bacc
    import concourse.mybir as mybir
    import concourse.tile as tile
    from concourse.masks import make_identity

    dt = mybir.dt
    mmdt = dt.float32r if use_f32r else dt.float32

    def mm_in(ap):
        return ap.bitcast(dt.float32r) if use_f32r else ap

    SUP = 256  # token super-tile
    n_sup = tokpc // SUP
    MT = SUP // 128  # m-tiles per super

    nc = bacc.Bacc(None, target_bir_lowering=False, debug=False,
                   enable_asserts=False, num_devices=1, num_swdge_queues=4)

    xt_in = nc.dram_tensor("xt", [IN_F, tokpc], dt.float32, kind="ExternalInput").ap()
    w_in = nc.dram_tensor("w", [OUT_F, IN_F], dt.float32, kind="ExternalInput").ap()
    b_in = nc.dram_tensor("b", [OUT_F, 1], dt.float32, kind="ExternalInput").ap()
    oft_in = nc.dram_tensor("oft", [N_BLOCKS, N_ELEM], dt.float32, kind="ExternalInput").ap()
    emat_in = nc.dram_tensor("emat", [N_ELEM, BS * BS], dt.float32, kind="ExternalInput").ap()
    # forward perm_out as int32 [2048,1] for the tiny b scatter
    pout_in = nc.dram_tensor("pout", [OUT_F, 1], dt.int32, kind="ExternalInput").ap()
    # inverse perms as wrapped int16 gather indices: [128, 8*16]
    gout_in = nc.dram_tensor("gout", [128, 8 * NB], dt.int16, kind="ExternalInput").ap()
    gin_in = nc.dram_tensor("gin", [128, 8 * KB], dt.int16, kind="ExternalInput").ap()
    out_d = nc.dram_tensor("out", [tokpc, OUT_F], dt.float32, kind="ExternalOutput").ap()

    qflat_d = nc.dram_tensor("qflat_d", [N_BLOCKS, BS, BS], dt.float32).ap()
    hnat_d = nc.dram_tensor("hnat_d", [OUT_F, IN_F], dt.float32).ap()
    g2nat_d = nc.dram_tensor("g2nat_d", [IN_F, OUT_F],
                             dt.float32r if use_f32r else dt.float32).ap()
    b2_d = nc.dram_tensor("b2_d", [OUT_F, 1], dt.float32).ap()

    with tile.TileContext(nc) as tc:
        with tc.tile_pool(name="const", bufs=1) as const:
            ident = const.tile([128, 128], dt.float32)
            make_identity(nc, ident)
            gidx_out = const.tile([128, 8 * NB], dt.int16)
            nc.sync.dma_start(gidx_out[:], gout_in[:])
            gidx_in = const.tile([128, 8 * KB], dt.int16)
            nc.sync.dma_start(gidx_in[:], gin_in[:])

            # ---------------- Phase Q: Q_flat = vec^T @ E ----------------
            with tc.tile_pool(name="sbq", bufs=1) as sbq, \
                 tc.tile_pool(name="psq", bufs=1, space="PSUM") as psq:
                oft_t = sbq.tile([128, N_ELEM], dt.float32)
                nc.sync.dma_start(oft_t[:], oft_in[:])
                qps = psq.tile([128, BS * BS], dt.float32)
                CH = 124
                for c in range(4):
                    lo = c * CH
                    sz = min(CH, N_ELEM - lo)
                    tp = psq.tile([CH, 128], dt.float32, tag="tps")
                    nc.tensor.transpose(out=tp[:sz, :], in_=oft_t[:, lo:lo + sz],
                                        identity=ident[:])
                    vt = sbq.tile([CH, 128], dt.float32, tag="vt")
                    nc.vector.tensor_copy(out=vt[:sz, :], in_=tp[:sz, :])
                    et = sbq.tile([CH, BS * BS], dt.float32, tag="et")
                    nc.sync.dma_start(et[:sz, :], emat_in[lo:lo + sz, :])
                    for nh in range(2):
                        nc.tensor.matmul(out=qps[:, nh * 512:(nh + 1) * 512],
                                         lhsT=vt[:sz, :],
                                         rhs=et[:sz, nh * 512:(nh + 1) * 512],
                                         start=(c == 0), stop=(c == 3))
                qsb = sbq.tile([128, BS * BS], dt.float32)
                nc.vector.tensor_copy(out=qsb[:], in_=qps[:])
                nc.sync.dma_start(qflat_d[:].rearrange("p a b -> p (a b)"), qsb[:])

            # ---------------- Phase C: BD4 Q tiles + Cayley ----------------
            # quad q holds tiles g=4q..4q+3.
            # g<16 -> BD4(R_left[4g..4g+3]); g>=16 -> BD4(R_right^T) = BD4(R(-Q))
            with tc.tile_pool(name="rpool", bufs=8) as rpool, \
                 tc.tile_pool(name="rf32p", bufs=4) as rf32p:
                r_quads = []
                rf_quads = []
                with tc.tile_pool(name="bdqp", bufs=1) as bdqp, \
                     tc.tile_pool(name="sbc", bufs=2) as sbc, \
                     tc.tile_pool(name="psc", bufs=2, space="PSUM") as psc:
                    bdq_all = bdqp.tile([128, 32, 128], dt.float32)
                    nc.vector.memset(bdq_all[:], 0.0)
                    qview = qflat_d[:].rearrange("(g four) i j -> four i g j", four=4)
                    for r in range(4):
                        nc.sync.dma_start(
                            bdq_all[r * BS:(r + 1) * BS, :, r * BS:(r + 1) * BS],
                            qview[r])
                    def cayley_quad(q):
                        bdq4 = bdq_all[:, 4 * q:4 * q + 4, :]
                        neg = sbc.tile([128, 4, 128], dt.float32, tag="neg")
                        nc.vector.tensor_scalar_mul(out=neg[:], in0=bdq4, scalar1=-1.0)
                        p2ps = psc.tile([128, 4, 128], dt.float32, tag="p2ps")
                        for gg in range(4):
                            nc.tensor.matmul(out=p2ps[:, gg, :], lhsT=neg[:, gg, :],
                                             rhs=bdq4[:, gg, :], start=True, stop=True)
                        p2 = sbc.tile([128, 4, 128], dt.float32, tag="p2")
                        nc.vector.tensor_copy(out=p2[:], in_=p2ps[:])
                        p3ps = psc.tile([128, 4, 128], dt.float32, tag="p3ps")
                        for gg in range(4):
                            nc.tensor.matmul(out=p3ps[:, gg, :], lhsT=p2[:, gg, :],
                                             rhs=bdq4[:, gg, :], start=True, stop=True)
                        negp3 = sbc.tile([128, 4, 128], dt.float32, tag="negp3")
                        nc.vector.tensor_scalar_mul(out=negp3[:], in0=p3ps[:],
                                                    scalar1=-1.0)
                        p3 = sbc.tile([128, 4, 128], dt.float32, tag="p3")
                        nc.vector.tensor_copy(out=p3[:], in_=p3ps[:])
                        p4ps = psc.tile([128, 4, 128], dt.float32, tag="p4ps")
                        for gg in range(4):
                            nc.tensor.matmul(out=p4ps[:, gg, :], lhsT=negp3[:, gg, :],
                                             rhs=bdq4[:, gg, :], start=True, stop=True)
                        # R = I + 2*(Q + P2 + P3 + P4)   (q < 4)
                        # R = I + 2*(P2 + P4 - Q - P3)   (q >= 4: R(-Q))
                        t1 = sbc.tile([128, 4, 128], dt.float32, tag="t1")
                        nc.vector.tensor_add(out=t1[:], in0=p2[:], in1=p4ps[:])
                        t2 = sbc.tile([128, 4, 128], dt.float32, tag="t2")
                        nc.vector.tensor_add(out=t2[:], in0=bdq4, in1=p3[:])
                        t3 = sbc.tile([128, 4, 128], dt.float32, tag="t3")
                        op = mybir.AluOpType.add if q < 4 else mybir.AluOpType.subtract
                        nc.vector.tensor_tensor(out=t3[:], in0=t1[:], in1=t2[:], op=op)
                        nc.vector.tensor_scalar_mul(out=t3[:], in0=t3[:], scalar1=2.0)
                        rq = rpool.tile([128, 4, 128], mmdt, tag="rq", name=f"rq_{q}")
                        for gg in range(4):
                            nc.vector.tensor_add(out=rq[:, gg, :], in0=t3[:, gg, :],
                                                 in1=ident[:])
                        r_quads.append(rq)
                        if q < 4:
                            rf = rf32p.tile([128, 4, 128], dt.float32, tag="rf",
                                            name=f"rf_{q}")
                            for gg in range(4):
                                nc.vector.tensor_add(out=rf[:, gg, :], in0=t3[:, gg, :],
                                                     in1=ident[:])
                            rf_quads.append(rf)

                    for q in range(4):
                        cayley_quad(q)

                    # Phase B here: rf quads ready; its Pool desc-gen drains
                    # during the remaining Cayley + H phases instead of
                    # delaying the critical T-phase gathers.
                    with tc.tile_pool(name="sbb", bufs=1) as sbb, \
                         tc.tile_pool(name="psb", bufs=1, space="PSUM") as psb:
                        b_sb = sbb.tile([128, NB], dt.float32)
                        nc.sync.dma_start(
                            b_sb[:], b_in[:].rearrange("(g p) one -> p (g one)", p=128))
                        pidx_all = sbb.tile([128, NB], dt.int32)
                        nc.sync.dma_start(
                            pidx_all[:],
                            pout_in[:].rearrange("(g p) one -> p (g one)", p=128))
                        brot = sbb.tile([128, NB], dt.float32)
                        for g in range(NB):
                            bps = psb.tile([128, 1], dt.float32, tag="bps")
                            nc.tensor.matmul(
                                out=bps[:], lhsT=rf_quads[g // 4][:, g % 4, :],
                                rhs=b_sb[:, g:g + 1], start=True, stop=True)
                            nc.vector.tensor_copy(out=brot[:, g:g + 1], in_=bps[:])
                        for g in range(NB):
                            nc.gpsimd.indirect_dma_start(
                                out=b2_d[:], out_offset=bass.IndirectOffsetOnAxis(
                                    ap=pidx_all[:, g:g + 1], axis=0),
                                in_=brot[:, g:g + 1], in_offset=None)

                    for q in range(4, 8):
                        cayley_quad(q)

                def r_tile(g):
                    return r_quads[g // 4][:, g % 4, :]

                def rf_tile(g):
                    return rf_quads[g // 4][:, g % 4, :]


                # ---------- Phase H: H = BD_L^T @ W, plain store ----------
                with tc.tile_pool(name="sbh", bufs=3) as sbh, \
                     tc.tile_pool(name="psh", bufs=2, space="PSUM") as psh:
                    for g in range(NB):
                        wt = sbh.tile([128, IN_F], mmdt, tag="wt")
                        nc.sync.dma_start(wt[:], mm_in(w_in[g * 128:(g + 1) * 128, :]))
                        hps = psh.tile([128, IN_F], dt.float32, tag="hps")
                        for n in range(IN_F // 512):
                            nc.tensor.matmul(out=hps[:, n * 512:(n + 1) * 512],
                                             lhsT=r_tile(g),
                                             rhs=wt[:, n * 512:(n + 1) * 512],
                                             start=True, stop=True)
                        hsb = sbh.tile([128, IN_F], dt.float32, tag="hsb")
                        if g % 3 < 2:
                            nc.vector.tensor_copy(out=hsb[:], in_=hps[:])
                        else:
                            nc.scalar.copy(out=hsb[:], in_=hps[:])
                        nc.sync.dma_start(hnat_d[g * 128:(g + 1) * 128, :], hsb[:])

                # --- Phase T: gather rows by inv_perm_out, transpose, G2 ---
                with tc.tile_pool(name="h2tp", bufs=KB) as h2tp, \
                     tc.tile_pool(name="sbt", bufs=2) as sbt, \
                     tc.tile_pool(name="pst", bufs=4, space="PSUM") as pst, \
                     tc.tile_pool(name="psg", bufs=1, space="PSUM") as psg:
                    h2t = []
                    for _i in range(KB):
                        h2t_i = h2tp.tile([128, OUT_F], mmdt, tag="h2t",
                                          name=f"h2t_{_i}")
                        h2t.append(h2t_i)
                    for gq in range(NB // 4):  # 4 row-blocks per group
                        rows = []
                        for c2 in range(2):
                            gc = gq * 2 + c2
                            h2row = sbt.tile([128, 2, IN_F], dt.float32, tag="h2row",
                                             name=f"h2row_{gc}")
                            nc.gpsimd.dma_gather(
                                out_ap=h2row[:], in_ap=hnat_d[:],
                                idxs_ap=gidx_out[:, gc * 16:(gc + 1) * 16],
                                num_idxs=256, num_idxs_reg=256, elem_size=IN_F,
                                queue_num=qsel(gc))
                            rows.append(h2row)
                        for i in range(KB):
                            tq = pst.tile([128, 4, 128], dt.float32, tag="ttp")
                            for j in range(4):
                                nc.tensor.transpose(
                                    out=tq[:, j, :],
                                    in_=rows[j // 2][:, j % 2, i * 128:(i + 1) * 128],
                                    identity=ident[:])
                            if (gq * KB + i) % 3 < 2:
                                nc.vector.tensor_copy(
                                    out=h2t[i][:, gq * 512:(gq + 1) * 512], in_=tq[:])
                            else:
                                nc.scalar.copy(
                                    out=h2t[i][:, gq * 512:(gq + 1) * 512], in_=tq[:])
                    for i in range(KB):
                        gps = psg.tile([128, OUT_F], dt.float32, tag="gps")
                        for n in range(OUT_F // 512):
                            nc.tensor.matmul(out=gps[:, n * 512:(n + 1) * 512],
                                             lhsT=r_tile(16 + i),
                                             rhs=h2t[i][:, n * 512:(n + 1) * 512],
                                             start=True, stop=True)
                        gsb = sbt.tile([128, OUT_F],
                                       dt.float32r if use_f32r else dt.float32,
                                       tag="gsb")
                        if i % 3 < 2:
                            nc.vector.tensor_copy(out=gsb[:], in_=gps[:])
                        else:
                            nc.scalar.copy(out=gsb[:], in_=gps[:])
                        nc.sync.dma_start(g2nat_d[i * 128:(i + 1) * 128, :], gsb[:])


            # ---------------- Phase G: the main GEMM ----------------
            with tc.tile_pool(name="biasp", bufs=1) as biasp:
                with tc.tile_pool(name="sbias", bufs=1) as sbias, \
                     tc.tile_pool(name="psbias", bufs=1, space="PSUM") as psbias:
                    b2row = sbias.tile([1, OUT_F], dt.float32)
                    nc.sync.dma_start(b2row[:1, :], b2_d[:].rearrange("a b -> b a"))
                    ones = sbias.tile([1, 128], dt.float32)
                    nc.vector.memset(ones[:], 1.0)
                    bbps = psbias.tile([128, OUT_F], dt.float32)
                    for n in range(OUT_F // 512):
                        nc.tensor.matmul(out=bbps[:, n * 512:(n + 1) * 512],
                                         lhsT=ones[:1, :],
                                         rhs=b2row[:1, n * 512:(n + 1) * 512],
                                         start=True, stop=True)
                    bias_sb = biasp.tile([128, OUT_F], dt.float32)
                    nc.vector.tensor_copy(out=bias_sb[:], in_=bbps[:])

                with tc.tile_pool(name="wfp", bufs=KB // 2) as wfp, \
                     tc.tile_pool(name="sbg", bufs=2) as sbg, \
                     tc.tile_pool(name="osbp", bufs=2) as osbp, \
                     tc.tile_pool(name="psgm", bufs=2, space="PSUM") as psgm:
                    weff2 = []
                    for _k in range(KB // 2):
                        weff_k = wfp.tile([128, 2, OUT_F], mmdt, tag="weff",
                                          name=f"weff_{_k}")
                        weff2.append(weff_k)
                    for kc in range(KB // 2):
                        nc.gpsimd.dma_gather(
                            out_ap=weff2[kc][:], in_ap=g2nat_d[:],
                            idxs_ap=gidx_in[:, kc * 16:(kc + 1) * 16],
                            num_idxs=256, num_idxs_reg=256, elem_size=OUT_F,
                            queue_num=qsel(kc))

                    xt_view = xt_in[:].rearrange("(k p) t -> p k t", p=128)
                    for s in range(n_sup):
                        xts = sbg.tile([128, KB, SUP], mmdt, tag="xts")
                        nc.sync.dma_start(
                            xts[:], mm_in(xt_view[:, :, s * SUP:(s + 1) * SUP]))
                        for mt in range(MT):
                            gps = psgm.tile([128, OUT_F], dt.float32, tag="gemmps")
                            for k in range(KB):
                                for n in range(OUT_F // 512):
                                    nc.tensor.matmul(
                                        out=gps[:, n * 512:(n + 1) * 512],
                                        lhsT=xts[:, k, mt * 128:(mt + 1) * 128],
                                        rhs=weff2[k // 2][:, k % 2, n * 512:(n + 1) * 512],
                                        start=(k == 0), stop=(k == KB - 1))
                            osb = osbp.tile([128, OUT_F], dt.float32, tag="osb")
                            nc.vector.tensor_add(out=osb[:], in0=gps[:], in1=bias_sb[:])
                            row0 = s * SUP + mt * 128
                            nc.sync.dma_start(out_d[row0:row0 + 128, :], osb[:])

    nc.compile()
    return nc


def _wrap_idx16(idx):
    """Pack N gather indices into dma_gather's wrapped layout: index j at
    [j % 16, j // 16], replicated across the 8 Q7 cores -> [128, N//16]."""
    n = len(idx)
    arr = np.zeros((16, n // 16), np.int16)
    j = np.arange(n)
    arr[j % 16, j // 16] = idx.astype(np.int16)
    return np.tile(arr, (8, 1))


def _host_prep(inputs):
    rows = np.asarray(inputs["rows"]).astype(np.int64)
    cols = np.asarray(inputs["cols"]).astype(np.int64)
    emat = np.zeros((N_ELEM, BS * BS), dtype=np.float32)
    e_idx = np.arange(N_ELEM)
    emat[e_idx, rows * BS + cols] = 1.0
    emat[e_idx, cols * BS + rows] = -1.0
    oft = np.concatenate([np.asarray(inputs["oft_L"], dtype=np.float32),
                          np.asarray(inputs["oft_R"], dtype=np.float32)], axis=0)
    pout = np.asarray(inputs["perm_out"]).astype(np.int32).reshape(OUT_F, 1)
    inv_pout = np.asarray(inputs["inv_perm_out"]).astype(np.int64)
    inv_pin = np.asarray(inputs["inv_perm_in"]).astype(np.int64)
    gout = np.concatenate([_wrap_idx16(inv_pout[gc * 256:(gc + 1) * 256])
                           for gc in range(NB // 2)], axis=1)
    gin = np.concatenate([_wrap_idx16(inv_pin[kc * 256:(kc + 1) * 256])
                          for kc in range(KB // 2)], axis=1)
    w = np.ascontiguousarray(np.asarray(inputs["W"], dtype=np.float32))
    b = np.asarray(inputs["b"], dtype=np.float32).reshape(OUT_F, 1)
    return emat, oft, pout, gout, gin, w, b


def _in_map(inputs):
    emat, oft, pout, gout, gin, w, b = _host_prep(inputs)
    return {"w": w, "b": b, "oft": oft, "emat": emat,
            "pout": pout, "gout": gout, "gin": gin}


def kernel(**inputs):
    from concourse.bass_utils import run_bass_kernel_spmd

    key = ("full", TOKPC)
    if key not in _CACHE:
        _CACHE[key] = _build(TOKPC)
    nc = _CACHE[key]

    x = np.asarray(inputs["x"], dtype=np.float32).reshape(TOKENS, IN_F)
    base = _in_map(inputs)
    in_maps = []
    for c in range(N_CORES):
        m = dict(base)
        m["xt"] = np.ascontiguousarray(x[c * TOKPC:(c + 1) * TOKPC].T)
        in_maps.append(m)

    res = run_bass_kernel_spmd(nc, in_maps, core_ids=list(range(N_CORES)))
    out = np.concatenate([res.results[c]["out"] for c in range(N_CORES)], axis=0)
    return out.reshape(4, 8192, OUT_F)



# revision 22
# speedup vs baseline: 46.6639x; 46.6639x over previous
"""TRN2 Bass kernel for nn_OFTLinear (forward).

Math: the whole OFT chain is linear, so
    out = x @ W_eff + b_eff
with
    W_eff = P_in . BD(R_right) . W^T . BD(R_left) . P_out      [2048 x 2048]
    b_eff = (b^T . BD(R_left))[inv_perm_out]
where R = Cayley-Neumann(skew(oft)) per 32x32 block, BD() is block-diagonal,
and P_in/P_out are the input/output feature permutations.

Host does layout-only work (same class of prep as the original x transpose):
shard x along tokens, apply the input feature permutation while transposing
each shard into a DMA-contiguous super-tile layout (fp32 DMA transpose is
unsupported on this stack, and a fused gather+transpose costs the same as
the transpose alone), cast x/W to bf16, pre-swizzle W into the column-strip
layout the device consumes, concat+transpose oft_L/oft_R, and build integer
index / one-hot constants.

Device pipeline (replicated on all 8 cores; x sharded along tokens).
All W_eff arithmetic stays on device and entirely in SBUF (no DRAM round
trips; the output feature permutation is an in-SBUF free-axis column gather
on gpsimd):
  Q:  Q_flat = oft^T-chunks @ E-chunks accumulated on PE (E: host-built
      one-hot skew-scatter matrix), no on-device transposes
  C:  BD4 tiles of Q (4 blocks per 128x128 tile) -> Cayley powers on PE.
      Skew-symmetry lets every power come from unnegated operands:
      p2 = Q^T Q = -Q^2, p3 = p2^T Q = -Q^3, p4 = p2^T p2 = Q^4, so
      R(+-Q) = I + 2*(+-(Q - p2ps) - (p3ps -+ p4ps)) needs no negation
      pass. Quads 0-3 (R_left) are computed first, then the first M strips
      are issued, then quads 4-7 (R_right^T = R(-Q)) overlap M's gathers.
  M:  per c-tile i: M_i = W[:,ci].T @ BD(R_left) strip-by-strip on PE (bf16),
      ap_gather columns by inv_perm_out, W2_i = BD4(R_right)(i) @ gathered_i
      -> bf16 weff tiles kept in SBUF. G2 of an earlier tile is issued after
      the strips of tile i so PE never stalls on the gpsimd gather.
  B:  bias row b^T BD_L on PE, broadcast via ones-outer-product, column
      gather by inv_perm_out (after M, PSUM shared with M's pool)
  G:  main GEMM: out = xT.T @ weff + bias, bf16 operands, fp32 PSUM accum.
      x/W loads ride the SP HWDGE ring (prefetched from instruction 0), the
      Q->C DRAM round trip and out stores ride the ACT ring.
"""

import numpy as np

IN_F = 2048
OUT_F = 2048
BS = 32
N_ELEM = BS * (BS - 1) // 2  # 496
N_BLOCKS = 128  # 64 left + 64 right
N_CORES = 8
TOKENS = 4 * 8192
TOKPC = TOKENS // N_CORES  # 4096
KB = IN_F // 128  # 16 k-blocks
NB = OUT_F // 128  # 16 n-blocks
ECH = 124  # oft chunk (496 = 4 * 124)

_CACHE = {}


def _build(tokpc):
    import concourse.bass as bass
    import concourse.bacc as bacc
    import concourse.mybir as mybir
    import concourse.tile as tile
    from concourse.masks import make_identity

    dt = mybir.dt
    f32r = dt.float32r

    def r32(ap):
        return ap.bitcast(f32r)

    SUP = 256  # token super-tile
    n_sup = tokpc // SUP
    MT = SUP // 128  # m-tiles per super
    XPRE = 2  # xts tiles prefetched before the preamble
    WPRE = 2  # wcol pair-tiles prefetched before the preamble

    nc = bacc.Bacc(None, target_bir_lowering=False, debug=False,
                   enable_asserts=False, num_devices=1, num_swdge_queues=1)

    # x: [128, s, k, t] = xp[s*256+t, k*128+p], flat [128, 16*16*256]
    xt_in = nc.dram_tensor("xt", [128, tokpc * KB], dt.bfloat16,
                           kind="ExternalInput").ap()
    # W: [128, ip, g, c'] = W[g*128+p, ip*256+c'], flat [128, 8*16*256]
    w_in = nc.dram_tensor("w", [128, IN_F * NB], dt.bfloat16,
                          kind="ExternalInput").ap()
    b_in = nc.dram_tensor("b", [OUT_F, 1], dt.float32, kind="ExternalInput").ap()
    # oft^T chunks: [124, c*128+g] = oft[g, c*124+e']
    oftt_in = nc.dram_tensor("oftt", [ECH, 4 * 128], dt.bfloat16,
                             kind="ExternalInput").ap()
    # emat chunks: [124, c*1024+ij] = emat[c*124+e', ij]
    emat_in = nc.dram_tensor("emat", [ECH, 4 * BS * BS], dt.bfloat16,
                             kind="ExternalInput").ap()
    # inv_perm_out as wrapped int16 gather indices for ap_gather: [128, 128]
    gout_in = nc.dram_tensor("gout", [128, OUT_F // 16], dt.int16,
                             kind="ExternalInput").ap()
    out_d = nc.dram_tensor("out", [tokpc, OUT_F], dt.float32,
                           kind="ExternalOutput").ap()

    qflat_d = nc.dram_tensor("qflat_d", [N_BLOCKS, BS, BS], dt.float32).ap()

    with tile.TileContext(nc) as tc:
        with tc.tile_pool(name="const", bufs=1) as const:
            ident = const.tile([128, 128], dt.float32)
            make_identity(nc, ident)
            # [128, 4, 128] broadcast copy of I (one-op R assembly tail)
            ident4 = const.tile([128, 4, 128], dt.float32)
            for gg in range(4):
                nc.vector.tensor_copy(out=ident4[:, gg, :], in_=ident[:])
            gidx_out = const.tile([128, OUT_F // 16], dt.int16)
            nc.sync.dma_start(gidx_out[:], gout_in[:])
            b_sb = const.tile([128, NB], f32r)
            nc.sync.dma_start(
                b_sb[:], r32(b_in[:].rearrange("(g p) one -> p (g one)", p=128)))

            # persistent pools (manual LIFO release at the end)
            bias_pool = tc.alloc_tile_pool(name="biasp", bufs=1)
            weff_pool = tc.alloc_tile_pool(name="weffp", bufs=1)
            sbg = tc.alloc_tile_pool(name="sbg", bufs=XPRE)
            wcolp = tc.alloc_tile_pool(name="wcolp", bufs=WPRE)

            # prefetch x super-tiles and W column-strips on the SP ring; the
            # Q->C critical DRAM chain rides the otherwise-idle ACT ring
            xts_tiles = {}

            def load_xts(s):
                t = sbg.tile([128, KB, SUP], dt.bfloat16, tag="xts",
                             name=f"xts_{s}")
                nc.sync.dma_start(
                    t[:].rearrange("p k t -> p (k t)"),
                    xt_in[:, s * KB * SUP:(s + 1) * KB * SUP])
                xts_tiles[s] = t

            w_view = w_in[:].rearrange("p (ip g c) -> p ip g c",
                                       ip=KB // 2, g=NB)
            wcol_tiles = {}

            def load_wcol(ip):
                t = wcolp.tile([128, NB, 256], dt.bfloat16, tag="wcol",
                               name=f"wcol_{ip}")
                nc.sync.dma_start(t[:], w_view[:, ip])
                wcol_tiles[ip] = t

            # ---------------- Phase Q: Q_flat = oft^T-chunks @ E ----------
            with tc.tile_pool(name="sbq", bufs=1) as sbq, \
                 tc.tile_pool(name="psq", bufs=1, space="PSUM") as psq:
                oftt = sbq.tile([ECH, 4 * 128], dt.bfloat16)
                nc.scalar.dma_start(oftt[:], oftt_in[:])
                ech = sbq.tile([ECH, 4, BS * BS], dt.bfloat16)
                nc.sync.dma_start(
                    ech[:].rearrange("p c e -> p (c e)"), emat_in[:])
                for _s in range(XPRE):
                    load_xts(_s)
                for _ip in range(WPRE):
                    load_wcol(_ip)
                qps = psq.tile([128, BS * BS], dt.float32)
                for c in range(4):
                    for nh in range(2):
                        nc.tensor.matmul(
                            out=qps[:, nh * 512:(nh + 1) * 512],
                            lhsT=oftt[:, c * 128:(c + 1) * 128],
                            rhs=ech[:, c, nh * 512:(nh + 1) * 512],
                            start=(c == 0), stop=(c == 3))
                qsb = sbq.tile([128, BS * BS], dt.float32)
                nc.vector.tensor_copy(out=qsb[:], in_=qps[:])
                nc.scalar.dma_start(qflat_d[:].rearrange("p a b -> p (a b)"),
                                    qsb[:])

            # ---------------- Phase C: BD4 Q tiles + Cayley ----------------
            # BD4 tile t (of 32) holds blocks 4t..4t+3 on its diagonal.
            # t<16 -> R_left tiles R(Q); t>=16 -> R_right^T tiles = R(-Q).
            with tc.tile_pool(name="rpool", bufs=8) as rpool, \
                 tc.tile_pool(name="rbfp", bufs=8) as rbfp:
                r_quads = {}
                rbf_quads = {}
                with tc.tile_pool(name="bdqp", bufs=1) as bdqp, \
                     tc.tile_pool(name="sbc", bufs=4) as sbc, \
                     tc.tile_pool(name="psc", bufs=8, space="PSUM") as psc:
                    bdq_all = bdqp.tile([128, 32, 128], dt.float32)
                    nc.vector.memset(bdq_all[:], 0.0)
                    qview = qflat_d[:].rearrange("(g four) i j -> four i g j",
                                                 four=4)
                    for r in range(4):
                        nc.scalar.dma_start(
                            bdq_all[r * BS:(r + 1) * BS, :, r * BS:(r + 1) * BS],
                            qview[r])

                    def cayley_batch(qs):
                        bdq4 = {q: bdq_all[:, 4 * q:4 * q + 4, :] for q in qs}
                        p2ps, p2, p3ps, p4ps = {}, {}, {}, {}
                        for q in qs:
                            p2ps[q] = psc.tile([128, 4, 128], dt.float32,
                                               tag="pps", name=f"p2ps_{q}")
                            for gg in range(4):
                                nc.tensor.matmul(out=p2ps[q][:, gg, :],
                                                 lhsT=bdq_all[:, 4 * q + gg, :],
                                                 rhs=bdq4[q][:, gg, :],
                                                 start=True, stop=True)
                        for q in qs:
                            p2[q] = sbc.tile([128, 4, 128], dt.float32,
                                             tag="p2", name=f"p2_{q}")
                            if q % 2 == 0:
                                nc.vector.tensor_copy(out=p2[q][:], in_=p2ps[q][:])
                            else:
                                nc.scalar.copy(out=p2[q][:], in_=p2ps[q][:])
                        for q in qs:
                            p3ps[q] = psc.tile([128, 4, 128], dt.float32,
                                               tag="pps", name=f"p3ps_{q}")
                            for gg in range(4):
                                nc.tensor.matmul(out=p3ps[q][:, gg, :],
                                                 lhsT=p2[q][:, gg, :],
                                                 rhs=bdq4[q][:, gg, :],
                                                 start=True, stop=True)
                        for q in qs:
                            p4ps[q] = psc.tile([128, 4, 128], dt.float32,
                                               tag="pps", name=f"p4ps_{q}")
                            for gg in range(4):
                                nc.tensor.matmul(out=p4ps[q][:, gg, :],
                                                 lhsT=p2[q][:, gg, :],
                                                 rhs=p2[q][:, gg, :],
                                                 start=True, stop=True)
                        for q in qs:
                            # p2ps = -Q^2, p3ps = -Q^3, p4ps = Q^4; one PSUM
                            # operand per DVE op (NCC_IBVF027)
                            # left : S = ((Q - p2ps) - p3ps) + p4ps
                            # right: S = (p3ps - (Q + p2ps)) + p4ps
                            left = q < 4
                            aa = sbc.tile([128, 4, 128], dt.float32, tag="tmp",
                                          name=f"aa_{q}", bufs=8)
                            nc.vector.tensor_tensor(
                                out=aa[:], in0=bdq4[q], in1=p2ps[q][:],
                                op=(mybir.AluOpType.subtract if left
                                    else mybir.AluOpType.add))
                            cc = sbc.tile([128, 4, 128], dt.float32, tag="tmp",
                                          name=f"cc_{q}", bufs=8)
                            if left:
                                nc.vector.tensor_sub(out=cc[:], in0=aa[:],
                                                     in1=p3ps[q][:])
                            else:
                                nc.vector.tensor_sub(out=cc[:], in0=p3ps[q][:],
                                                     in1=aa[:])
                            t3 = sbc.tile([128, 4, 128], dt.float32, tag="tmp",
                                          name=f"t3_{q}", bufs=8)
                            nc.vector.tensor_add(out=t3[:], in0=cc[:],
                                                 in1=p4ps[q][:])
                            rq = rpool.tile([128, 4, 128], f32r, tag="rq",
                                            name=f"rq_{q}")
                            # rq = (t3 * 2) + I
                            nc.vector.scalar_tensor_tensor(
                                out=rq[:], in0=t3[:], scalar=2.0,
                                in1=ident4[:], op0=mybir.AluOpType.mult,
                                op1=mybir.AluOpType.add)
                            r_quads[q] = rq
                            rbf = rbfp.tile([128, 4, 128], dt.bfloat16,
                                            tag="rbf", name=f"rbf_{q}")
                            if q % 2 == 0:
                                nc.scalar.copy(out=rbf[:],
                                               in_=rq[:].bitcast(dt.float32))
                            else:
                                nc.vector.tensor_copy(
                                    out=rbf[:], in_=rq[:].bitcast(dt.float32))
                            rbf_quads[q] = rbf

                    def r_tile(t):
                        return r_quads[t // 4][:, t % 4, :]

                    def rbf_tile(t):
                        return rbf_quads[t // 4][:, t % 4, :]

                    for qb in range(4):
                        cayley_batch([2 * qb, 2 * qb + 1])

                # psc/sbc/bdqp closed; PSUM now free for M's pipeline pool
                h2tp = tc.alloc_tile_pool(name="h2tp", bufs=2)
                h2gp = tc.alloc_tile_pool(name="h2gp", bufs=2)
                h2gbp = tc.alloc_tile_pool(name="h2gbp", bufs=3)
                with tc.tile_pool(name="psm", bufs=2, space="PSUM") as psm:
                    # ------- Phase M: weff tiles, all in SBUF -------
                    # per c-tile i:
                    #   M_i[c, o'] = sum_o W[o, c] BD_L[o, o']  (strip / o-tile)
                    #   Mg_i = ap_gather(M_i, inv_perm_out)     (free-axis cols)
                    #   weff_i = BD4(R_right)(i) @ Mg_i         -> bf16
                    weff = {}
                    pending = []

                    def issue_strips(i):
                        wcol = wcol_tiles[i // 2]
                        ih = i % 2
                        mps = psm.tile([128, OUT_F], dt.float32, tag="mps",
                                       name=f"mps_{i}")
                        for g in range(NB):
                            nc.tensor.matmul(
                                out=mps[:, g * 128:(g + 1) * 128],
                                lhsT=wcol[:, g, ih * 128:(ih + 1) * 128],
                                rhs=rbf_tile(g), start=True, stop=True)
                        h2t = h2tp.tile([128, OUT_F], dt.float32, tag="h2t",
                                        name=f"h2t_{i}")
                        if i % 2 == 0:
                            nc.vector.tensor_copy(out=h2t[:], in_=mps[:])
                        else:
                            nc.scalar.copy(out=h2t[:], in_=mps[:])
                        h2g = h2gp.tile([128, OUT_F], dt.float32, tag="h2g",
                                        name=f"h2g_{i}")
                        nc.gpsimd.ap_gather(out_ap=h2g[:], in_ap=h2t[:],
                                            idxs_ap=gidx_out[:], channels=128,
                                            num_elems=OUT_F, d=1,
                                            num_idxs=OUT_F)
                        h2gb = h2gbp.tile([128, OUT_F], dt.bfloat16, tag="h2gb",
                                          name=f"h2gb_{i}")
                        if i % 2 == 0:
                            nc.scalar.copy(out=h2gb[:], in_=h2g[:])
                        else:
                            nc.vector.tensor_copy(out=h2gb[:], in_=h2g[:])
                        pending.append((i, h2gb))

                    def issue_g2():
                        i, h2gb_i = pending.pop(0)
                        gps = psm.tile([128, OUT_F], dt.float32, tag="mps",
                                       name=f"g2ps_{i}")
                        for n in range(OUT_F // 512):
                            nc.tensor.matmul(
                                out=gps[:, n * 512:(n + 1) * 512],
                                lhsT=rbf_tile(16 + i),
                                rhs=h2gb_i[:, n * 512:(n + 1) * 512],
                                start=True, stop=True)
                        weff_i = weff_pool.tile([128, OUT_F], dt.bfloat16,
                                                tag=f"weff{i}", name=f"weff_{i}")
                        if i % 2 == 0:
                            nc.scalar.copy(out=weff_i[:], in_=gps[:])
                        else:
                            nc.vector.tensor_copy(out=weff_i[:], in_=gps[:])
                        weff[i] = weff_i

                    for i in range(KB):
                        if i // 2 + 1 < KB // 2 and (i // 2 + 1) not in wcol_tiles:
                            load_wcol(i // 2 + 1)
                        issue_strips(i)
                        if i >= 2:
                            issue_g2()
                    while pending:
                        issue_g2()

                    # ------------ Phase B: bias row + gather ------------
                    brow_ps = psm.tile([1, OUT_F], dt.float32, tag="mps",
                                       name="brow_ps")
                    for g in range(NB):
                        nc.tensor.matmul(out=brow_ps[:1, g * 128:(g + 1) * 128],
                                         lhsT=b_sb[:, g:g + 1],
                                         rhs=r_tile(g), start=True, stop=True)
                    brow_sb = bias_pool.tile([1, OUT_F], f32r, name="brow_sb")
                    nc.vector.tensor_copy(out=brow_sb[:], in_=brow_ps[:])
                    ones_f = bias_pool.tile([1, 128], dt.float32, name="ones_f")
                    nc.vector.memset(ones_f[:], 1.0)
                    ones1 = bias_pool.tile([1, 128], f32r, name="ones1")
                    nc.vector.tensor_copy(out=ones1[:], in_=ones_f[:])
                    bias_ps = psm.tile([128, OUT_F], dt.float32, tag="mps",
                                       name="bias_ps")
                    for n in range(OUT_F // 512):
                        nc.tensor.matmul(out=bias_ps[:, n * 512:(n + 1) * 512],
                                         lhsT=ones1[:1, :],
                                         rhs=brow_sb[:1, n * 512:(n + 1) * 512],
                                         start=True, stop=True)
                    bias_full = h2tp.tile([128, OUT_F], dt.float32,
                                          tag="h2t", name="bias_full")
                    nc.vector.tensor_copy(out=bias_full[:], in_=bias_ps[:])
                    bias_sb = bias_pool.tile([128, OUT_F], dt.float32,
                                             name="bias_sb")
                    nc.gpsimd.ap_gather(out_ap=bias_sb[:], in_ap=bias_full[:],
                                        idxs_ap=gidx_out[:], channels=128,
                                        num_elems=OUT_F, d=1, num_idxs=OUT_F)

                h2gbp.release()
                h2gp.release()
                h2tp.release()

            # ---------------- Phase G: the main GEMM ----------------
            with tc.tile_pool(name="osbp", bufs=3) as osbp, \
                 tc.tile_pool(name="psgm", bufs=2, space="PSUM") as psgm:
                for s in range(n_sup):
                    if s + XPRE < n_sup:
                        load_xts(s + XPRE)
                    xts = xts_tiles[s]
                    for mt in range(MT):
                        gps = psgm.tile([128, OUT_F], dt.float32, tag="gemmps",
                                        name=f"gps_{s}_{mt}")
                        for k in range(KB):
                            for n in range(OUT_F // 512):
                                nc.tensor.matmul(
                                    out=gps[:, n * 512:(n + 1) * 512],
                                    lhsT=xts[:, k, mt * 128:(mt + 1) * 128],
                                    rhs=weff[k][:, n * 512:(n + 1) * 512],
                                    start=(k == 0), stop=(k == KB - 1))
                        osb = osbp.tile([128, OUT_F], dt.float32, tag="osb",
                                        name=f"osb_{s}_{mt}")
                        nc.vector.tensor_add(out=osb[:], in0=gps[:],
                                             in1=bias_sb[:])
                        row0 = s * SUP + mt * 128
                        nc.scalar.dma_start(out_d[row0:row0 + 128, :], osb[:])

            wcolp.release()
            sbg.release()
            weff_pool.release()
            bias_pool.release()

    nc.compile()
    return nc


def _wrap_idx16(idx):
    """Pack N gather indices into the wrapped layout gpsimd expects: index j
    at [j % 16, j // 16], replicated across the 8 Q7 cores -> [128, N//16]."""
    n = len(idx)
    arr = np.zeros((16, n // 16), np.int16)
    j = np.arange(n)
    arr[j % 16, j // 16] = idx.astype(np.int16)
    return np.tile(arr, (8, 1))


def _host_prep(inputs):
    import ml_dtypes
    rows = np.asarray(inputs["rows"]).astype(np.int64)
    cols = np.asarray(inputs["cols"]).astype(np.int64)
    emat = np.zeros((N_ELEM, BS * BS), dtype=np.float32)
    e_idx = np.arange(N_ELEM)
    emat[e_idx, rows * BS + cols] = 1.0
    emat[e_idx, cols * BS + rows] = -1.0
    # [124, c*1024+ij] = emat[c*124+e', ij]
    emat4 = np.ascontiguousarray(
        emat.reshape(4, ECH, BS * BS).transpose(1, 0, 2)
        .reshape(ECH, -1)).astype(ml_dtypes.bfloat16)
    oft = np.concatenate([np.asarray(inputs["oft_L"], dtype=np.float32),
                          np.asarray(inputs["oft_R"], dtype=np.float32)], axis=0)
    # [124, c*128+g] = oft[g, c*124+e']
    oftt = np.ascontiguousarray(
        oft.reshape(128, 4, ECH).transpose(2, 1, 0)
        .reshape(ECH, -1)).astype(ml_dtypes.bfloat16)
    inv_pout = np.asarray(inputs["inv_perm_out"]).astype(np.int64)
    gout = _wrap_idx16(inv_pout)
    # W -> [p, ip, g, c'] = W[g*128+p, ip*256+c'], bf16
    w = np.asarray(inputs["W"], dtype=np.float32)
    wsw = np.ascontiguousarray(
        w.reshape(NB, 128, KB // 2, 256).transpose(1, 2, 0, 3)
        .reshape(128, -1)).astype(ml_dtypes.bfloat16)
    b = np.asarray(inputs["b"], dtype=np.float32).reshape(OUT_F, 1)
    return emat4, oftt, gout, wsw, b


def _in_map(inputs):
    emat4, oftt, gout, wsw, b = _host_prep(inputs)
    return {"w": wsw, "b": b, "oftt": oftt, "emat": emat4, "gout": gout}


def _shard_x(x, perm_in, core):
    """[128, s*4096 + k*256 + t] = xp[core*4096 + s*256 + t, k*128 + p], bf16."""
    import ml_dtypes
    xs = x[core * TOKPC:(core + 1) * TOKPC][:, perm_in]
    x4 = xs.reshape(TOKPC // 256, 256, KB, 128).transpose(3, 0, 2, 1)
    return np.ascontiguousarray(x4.reshape(128, -1).astype(ml_dtypes.bfloat16))


def kernel(**inputs):
    from concourse.bass_utils import run_bass_kernel_spmd

    key = ("full", TOKPC)
    if key not in _CACHE:
        _CACHE[key] = _build(TOKPC)
    nc = _CACHE[key]

    perm_in = np.asarray(inputs["perm_in"]).astype(np.int64)
    x = np.asarray(inputs["x"], dtype=np.float32).reshape(TOKENS, IN_F)
    base = _in_map(inputs)
    in_maps = []
    for c in range(N_CORES):
        m = dict(base)
        m["xt"] = _shard_x(x, perm_in, c)
        in_maps.append(m)

    res = run_bass_kernel_spmd(nc, in_maps, core_ids=list(range(N_CORES)))
    out = np.concatenate([res.results[c]["out"] for c in range(N_CORES)], axis=0)
    return out.reshape(4, 8192, OUT_F)


# revision 27
# speedup vs baseline: 87.4798x; 1.8747x over previous
"""TRN2 Bass kernel for nn_OFTLinear (forward).

Math: the whole OFT chain is linear, so
    out = x @ W_eff + b_eff
with
    W_eff = P_in . BD(R_right) . W^T . BD(R_left) . P_out      [2048 x 2048]
    b_eff = (BD(R_left)^T b)[inv_perm_out]
where R = Cayley-Neumann(skew(oft)) per 32x32 block, BD() is block-diagonal,
and P_in/P_out are the input/output feature permutations.

Device pipeline (replicated on all 8 cores; x sharded along tokens):
  Q:  Q_flat = vec^T @ E (E: host-built one-hot skew-scatter matrix)
  C:  BD4 tiles of Q (4 blocks per 128x128 tile) -> Cayley powers on PE ->
      R_left tiles (g<16) and R_right^T = R(-Q) tiles (g>=16)
  H:  H = BD(R_left)^T @ W on PE, plain-stored to DRAM [out, in]
  T:  h2row_g = row-gather of H by inv_perm_out (dma_gather, 4 SWDGE queues)
      -> PE-transpose into H2T tiles [in, out]
  G:  G2_i = BD(R_right) @ H2T_i on PE, plain-stored [in, out] (float32r)
  GEMM: W_eff k-tiles = row-gather of G2 by inv_perm_in; out = xT.T@W_eff + b
      (float32r matmuls, fp32 accumulate)

Host does layout-only work: shard x along tokens, transpose each shard
(fp32 DMA transpose is unsupported on this stack), concat oft_L/oft_R, and
build integer index/one-hot constants from the permutation/index buffers.
"""

import numpy as np

IN_F = 2048
OUT_F = 2048
BS = 32
N_ELEM = BS * (BS - 1) // 2  # 496
N_BLOCKS = 128  # 64 left + 64 right
N_CORES = 8
TOKENS = 4 * 8192
TOKPC = TOKENS // N_CORES  # 4096
KB = IN_F // 128  # 16 k-blocks
NB = OUT_F // 128  # 16 n-blocks

_CACHE = {}


def _build(tokpc, use_f32r=True):
    import os
    qmode = os.environ.get("GATHER_QMODE", "q0")
    if qmode == "q0":
        qsel = lambda j: 0
    elif qmode == "rr3":
        qsel = lambda j: 1 + (j % 3)
    else:
        qsel = lambda j: j % 4
    import concourse.bass as bass
    import concourse.bacc as bacc
    import concourse.mybir as mybir
    import concourse.tile as tile
    from concourse.masks import make_identity

    dt = mybir.dt
    mmdt = dt.float32r if use_f32r else dt.float32

    def mm_in(ap):
        return ap.bitcast(dt.float32r) if use_f32r else ap

    SUP = 256  # token super-tile
    n_sup = tokpc // SUP
    MT = SUP // 128  # m-tiles per super

    nc = bacc.Bacc(None, target_bir_lowering=False, debug=False,
                   enable_asserts=False, num_devices=1, num_swdge_queues=4)

    xt_in = nc.dram_tensor("xt", [IN_F, tokpc], dt.float32, kind="ExternalInput").ap()
    w_in = nc.dram_tensor("w", [OUT_F, IN_F], dt.float32, kind="ExternalInput").ap()
    b_in = nc.dram_tensor("b", [OUT_F, 1], dt.float32, kind="ExternalInput").ap()
    oft_in = nc.dram_tensor("oft", [N_BLOCKS, N_ELEM], dt.float32, kind="ExternalInput").ap()
    emat_in = nc.dram_tensor("emat", [N_ELEM, BS * BS], dt.float32, kind="ExternalInput").ap()
    # forward perm_out as int32 [2048,1] for the tiny b scatter
    pout_in = nc.dram_tensor("pout", [OUT_F, 1], dt.int32, kind="ExternalInput").ap()
    # inverse perms as wrapped int16 gather indices: [128, 8*16]
    gout_in = nc.dram_tensor("gout", [128, 8 * NB], dt.int16, kind="ExternalInput").ap()
    gin_in = nc.dram_tensor("gin", [128, 8 * KB], dt.int16, kind="ExternalInput").ap()
    out_d = nc.dram_tensor("out", [tokpc, OUT_F], dt.float32, kind="ExternalOutput").ap()

    qflat_d = nc.dram_tensor("qflat_d", [N_BLOCKS, BS, BS], dt.float32).ap()
    hnat_d = nc.dram_tensor("hnat_d", [OUT_F, IN_F], dt.float32).ap()
    g2nat_d = nc.dram_tensor("g2nat_d", [IN_F, OUT_F],
                             dt.float32r if use_f32r else dt.float32).ap()
    b2_d = nc.dram_tensor("b2_d", [OUT_F, 1], dt.float32).ap()

    with tile.TileContext(nc) as tc:
        with tc.tile_pool(name="const", bufs=1) as const:
            ident = const.tile([128, 128], dt.float32)
            make_identity(nc, ident)
            gidx_out = const.tile([128, 8 * NB], dt.int16)
            nc.sync.dma_start(gidx_out[:], gout_in[:])
            gidx_in = const.tile([128, 8 * KB], dt.int16)
            nc.sync.dma_start(gidx_in[:], gin_in[:])

            # ---------------- Phase Q: Q_flat = vec^T @ E ----------------
            with tc.tile_pool(name="sbq", bufs=1) as sbq, \
                 tc.tile_pool(name="psq", bufs=1, space="PSUM") as psq:
                oft_t = sbq.tile([128, N_ELEM], dt.float32)
                nc.sync.dma_start(oft_t[:], oft_in[:])
                qps = psq.tile([128, BS * BS], dt.float32)
                CH = 124
                for c in range(4):
                    lo = c * CH
                    sz = min(CH, N_ELEM - lo)
                    tp = psq.tile([CH, 128], dt.float32, tag="tps")
                    nc.tensor.transpose(out=tp[:sz, :], in_=oft_t[:, lo:lo + sz],
                                        identity=ident[:])
                    vt = sbq.tile([CH, 128], dt.float32, tag="vt")
                    nc.vector.tensor_copy(out=vt[:sz, :], in_=tp[:sz, :])
                    et = sbq.tile([CH, BS * BS], dt.float32, tag="et")
                    nc.sync.dma_start(et[:sz, :], emat_in[lo:lo + sz, :])
                    for nh in range(2):
                        nc.tensor.matmul(out=qps[:, nh * 512:(nh + 1) * 512],
                                         lhsT=vt[:sz, :],
                                         rhs=et[:sz, nh * 512:(nh + 1) * 512],
                                         start=(c == 0), stop=(c == 3))
                qsb = sbq.tile([128, BS * BS], dt.float32)
                nc.vector.tensor_copy(out=qsb[:], in_=qps[:])
                nc.sync.dma_start(qflat_d[:].rearrange("p a b -> p (a b)"), qsb[:])

            # ---------------- Phase C: BD4 Q tiles + Cayley ----------------
            # quad q holds tiles g=4q..4q+3.
            # g<16 -> BD4(R_left[4g..4g+3]); g>=16 -> BD4(R_right^T) = BD4(R(-Q))
            with tc.tile_pool(name="rpool", bufs=8) as rpool, \
                 tc.tile_pool(name="rf32p", bufs=4) as rf32p:
                r_quads = []
                rf_quads = []
                with tc.tile_pool(name="bdqp", bufs=1) as bdqp, \
                     tc.tile_pool(name="sbc", bufs=2) as sbc, \
                     tc.tile_pool(name="psc", bufs=2, space="PSUM") as psc:
                    bdq_all = bdqp.tile([128, 32, 128], dt.float32)
                    nc.vector.memset(bdq_all[:], 0.0)
                    qview = qflat_d[:].rearrange("(g four) i j -> four i g j", four=4)
                    for r in range(4):
                        nc.sync.dma_start(
                            bdq_all[r * BS:(r + 1) * BS, :, r * BS:(r + 1) * BS],
                            qview[r])
                    def cayley_quad(q):
                        bdq4 = bdq_all[:, 4 * q:4 * q + 4, :]
                        neg = sbc.tile([128, 4, 128], dt.float32, tag="neg")
                        nc.vector.tensor_scalar_mul(out=neg[:], in0=bdq4, scalar1=-1.0)
                        p2ps = psc.tile([128, 4, 128], dt.float32, tag="p2ps")
                        for gg in range(4):
                            nc.tensor.matmul(out=p2ps[:, gg, :], lhsT=neg[:, gg, :],
                                             rhs=bdq4[:, gg, :], start=True, stop=True)
                        p2 = sbc.tile([128, 4, 128], dt.float32, tag="p2")
                        nc.vector.tensor_copy(out=p2[:], in_=p2ps[:])
                        p3ps = psc.tile([128, 4, 128], dt.float32, tag="p3ps")
                        for gg in range(4):
                            nc.tensor.matmul(out=p3ps[:, gg, :], lhsT=p2[:, gg, :],
                                             rhs=bdq4[:, gg, :], start=True, stop=True)
                        negp3 = sbc.tile([128, 4, 128], dt.float32, tag="negp3")
                        nc.vector.tensor_scalar_mul(out=negp3[:], in0=p3ps[:],
                                                    scalar1=-1.0)
                        p3 = sbc.tile([128, 4, 128], dt.float32, tag="p3")
                        nc.vector.tensor_copy(out=p3[:], in_=p3ps[:])
                        p4ps = psc.tile([128, 4, 128], dt.float32, tag="p4ps")
                        for gg in range(4):
                            nc.tensor.matmul(out=p4ps[:, gg, :], lhsT=negp3[:, gg, :],
                                             rhs=bdq4[:, gg, :], start=True, stop=True)
                        # R = I + 2*(Q + P2 + P3 + P4)   (q < 4)
                        # R = I + 2*(P2 + P4 - Q - P3)   (q >= 4: R(-Q))
                        t1 = sbc.tile([128, 4, 128], dt.float32, tag="t1")
                        nc.vector.tensor_add(out=t1[:], in0=p2[:], in1=p4ps[:])
                        t2 = sbc.tile([128, 4, 128], dt.float32, tag="t2")
                        nc.vector.tensor_add(out=t2[:], in0=bdq4, in1=p3[:])
                        t3 = sbc.tile([128, 4, 128], dt.float32, tag="t3")
                        op = mybir.AluOpType.add if q < 4 else mybir.AluOpType.subtract
                        nc.vector.tensor_tensor(out=t3[:], in0=t1[:], in1=t2[:], op=op)
                        nc.vector.tensor_scalar_mul(out=t3[:], in0=t3[:], scalar1=2.0)
                        rq = rpool.tile([128, 4, 128], mmdt, tag="rq", name=f"rq_{q}")
                        for gg in range(4):
                            nc.vector.tensor_add(out=rq[:, gg, :], in0=t3[:, gg, :],
                                                 in1=ident[:])
                        r_quads.append(rq)
                        if q < 4:
                            rf = rf32p.tile([128, 4, 128], dt.float32, tag="rf",
                                            name=f"rf_{q}")
                            for gg in range(4):
                                nc.vector.tensor_add(out=rf[:, gg, :], in0=t3[:, gg, :],
                                                     in1=ident[:])
                            rf_quads.append(rf)

                    for q in range(4):
                        cayley_quad(q)

                    # Phase B here: rf quads ready; its Pool desc-gen drains
                    # during the remaining Cayley + H phases instead of
                    # delaying the critical T-phase gathers.
                    with tc.tile_pool(name="sbb", bufs=1) as sbb, \
                         tc.tile_pool(name="psb", bufs=1, space="PSUM") as psb:
                        b_sb = sbb.tile([128, NB], dt.float32)
                        nc.sync.dma_start(
                            b_sb[:], b_in[:].rearrange("(g p) one -> p (g one)", p=128))
                        pidx_all = sbb.tile([128, NB], dt.int32)
                        nc.sync.dma_start(
                            pidx_all[:],
                            pout_in[:].rearrange("(g p) one -> p (g one)", p=128))
                        brot = sbb.tile([128, NB], dt.float32)
                        for g in range(NB):
                            bps = psb.tile([128, 1], dt.float32, tag="bps")
                            nc.tensor.matmul(
                                out=bps[:], lhsT=rf_quads[g // 4][:, g % 4, :],
                                rhs=b_sb[:, g:g + 1], start=True, stop=True)
                            nc.vector.tensor_copy(out=brot[:, g:g + 1], in_=bps[:])
                        for g in range(NB):
                            nc.gpsimd.indirect_dma_start(
                                out=b2_d[:], out_offset=bass.IndirectOffsetOnAxis(
                                    ap=pidx_all[:, g:g + 1], axis=0),
                                in_=brot[:, g:g + 1], in_offset=None)

                    for q in range(4, 8):
                        cayley_quad(q)

                def r_tile(g):
                    return r_quads[g // 4][:, g % 4, :]

                def rf_tile(g):
                    return rf_quads[g // 4][:, g % 4, :]


                # ---------- Phase H: H = BD_L^T @ W, plain store ----------
                with tc.tile_pool(name="sbh", bufs=3) as sbh, \
                     tc.tile_pool(name="psh", bufs=2, space="PSUM") as psh:
                    for g in range(NB):
                        wt = sbh.tile([128, IN_F], mmdt, tag="wt")
                        nc.sync.dma_start(wt[:], mm_in(w_in[g * 128:(g + 1) * 128, :]))
                        hps = psh.tile([128, IN_F], dt.float32, tag="hps")
                        for n in range(IN_F // 512):
                            nc.tensor.matmul(out=hps[:, n * 512:(n + 1) * 512],
                                             lhsT=r_tile(g),
                                             rhs=wt[:, n * 512:(n + 1) * 512],
                                             start=True, stop=True)
                        hsb = sbh.tile([128, IN_F], dt.float32, tag="hsb")
                        if g % 3 < 2:
                            nc.vector.tensor_copy(out=hsb[:], in_=hps[:])
                        else:
                            nc.scalar.copy(out=hsb[:], in_=hps[:])
                        nc.sync.dma_start(hnat_d[g * 128:(g + 1) * 128, :], hsb[:])

                # --- Phase T: gather rows by inv_perm_out, transpose, G2 ---
                with tc.tile_pool(name="h2tp", bufs=KB) as h2tp, \
                     tc.tile_pool(name="sbt", bufs=2) as sbt, \
                     tc.tile_pool(name="pst", bufs=4, space="PSUM") as pst, \
                     tc.tile_pool(name="psg", bufs=1, space="PSUM") as psg:
                    h2t = []
                    for _i in range(KB):
                        h2t_i = h2tp.tile([128, OUT_F], mmdt, tag="h2t",
                                          name=f"h2t_{_i}")
                        h2t.append(h2t_i)
                    for gq in range(NB // 4):  # 4 row-blocks per group
                        rows = []
                        for c2 in range(2):
                            gc = gq * 2 + c2
                            h2row = sbt.tile([128, 2, IN_F], dt.float32, tag="h2row",
                                             name=f"h2row_{gc}")
                            nc.gpsimd.dma_gather(
                                out_ap=h2row[:], in_ap=hnat_d[:],
                                idxs_ap=gidx_out[:, gc * 16:(gc + 1) * 16],
                                num_idxs=256, num_idxs_reg=256, elem_size=IN_F,
                                queue_num=qsel(gc))
                            rows.append(h2row)
                        for i in range(KB):
                            tq = pst.tile([128, 4, 128], dt.float32, tag="ttp")
                            for j in range(4):
                                nc.tensor.transpose(
                                    out=tq[:, j, :],
                                    in_=rows[j // 2][:, j % 2, i * 128:(i + 1) * 128],
                                    identity=ident[:])
                            if (gq * KB + i) % 3 < 2:
                                nc.vector.tensor_copy(
                                    out=h2t[i][:, gq * 512:(gq + 1) * 512], in_=tq[:])
                            else:
                                nc.scalar.copy(
                                    out=h2t[i][:, gq * 512:(gq + 1) * 512], in_=tq[:])
                    for i in range(KB):
                        gps = psg.tile([128, OUT_F], dt.float32, tag="gps")
                        for n in range(OUT_F // 512):
                            nc.tensor.matmul(out=gps[:, n * 512:(n + 1) * 512],
                                             lhsT=r_tile(16 + i),
                                             rhs=h2t[i][:, n * 512:(n + 1) * 512],
                                             start=True, stop=True)
                        gsb = sbt.tile([128, OUT_F],
                                       dt.float32r if use_f32r else dt.float32,
                                       tag="gsb")
                        if i % 3 < 2:
                            nc.vector.tensor_copy(out=gsb[:], in_=gps[:])
                        else:
                            nc.scalar.copy(out=gsb[:], in_=gps[:])
                        nc.sync.dma_start(g2nat_d[i * 128:(i + 1) * 128, :], gsb[:])


            # ---------------- Phase G: the main GEMM ----------------
            with tc.tile_pool(name="biasp", bufs=1) as biasp:
                with tc.tile_pool(name="sbias", bufs=1) as sbias, \
                     tc.tile_pool(name="psbias", bufs=1, space="PSUM") as psbias:
                    b2row = sbias.tile([1, OUT_F], dt.float32)
                    nc.sync.dma_start(b2row[:1, :], b2_d[:].rearrange("a b -> b a"))
                    ones = sbias.tile([1, 128], dt.float32)
                    nc.vector.memset(ones[:], 1.0)
                    bbps = psbias.tile([128, OUT_F], dt.float32)
                    for n in range(OUT_F // 512):
                        nc.tensor.matmul(out=bbps[:, n * 512:(n + 1) * 512],
                                         lhsT=ones[:1, :],
                                         rhs=b2row[:1, n * 512:(n + 1) * 512],
                                         start=True, stop=True)
                    bias_sb = biasp.tile([128, OUT_F], dt.float32)
                    nc.vector.tensor_copy(out=bias_sb[:], in_=bbps[:])

                with tc.tile_pool(name="wfp", bufs=KB // 2) as wfp, \
                     tc.tile_pool(name="sbg", bufs=2) as sbg, \
                     tc.tile_pool(name="osbp", bufs=2) as osbp, \
                     tc.tile_pool(name="psgm", bufs=2, space="PSUM") as psgm:
                    weff2 = []
                    for _k in range(KB // 2):
                        weff_k = wfp.tile([128, 2, OUT_F], mmdt, tag="weff",
                                          name=f"weff_{_k}")
                        weff2.append(weff_k)
                    for kc in range(KB // 2):
                        nc.gpsimd.dma_gather(
                            out_ap=weff2[kc][:], in_ap=g2nat_d[:],
                            idxs_ap=gidx_in[:, kc * 16:(kc + 1) * 16],
                            num_idxs=256, num_idxs_reg=256, elem_size=OUT_F,
                            queue_num=qsel(kc))

                    xt_view = xt_in[:].rearrange("(k p) t -> p k t", p=128)
                    for s in range(n_sup):
                        xts = sbg.tile([128, KB, SUP], mmdt, tag="xts")
                        nc.sync.dma_start(
                            xts[:], mm_in(xt_view[:, :, s * SUP:(s + 1) * SUP]))
                        for mt in range(MT):
                            gps = psgm.tile([128, OUT_F], dt.float32, tag="gemmps")
                            for k in range(KB):
                                for n in range(OUT_F // 512):
                                    nc.tensor.matmul(
                                        out=gps[:, n * 512:(n + 1) * 512],
                                        lhsT=xts[:, k, mt * 128:(mt + 1) * 128],
                                        rhs=weff2[k // 2][:, k % 2, n * 512:(n + 1) * 512],
                                        start=(k == 0), stop=(k == KB - 1))
                            osb = osbp.tile([128, OUT_F], dt.float32, tag="osb")
                            nc.vector.tensor_add(out=osb[:], in0=gps[:], in1=bias_sb[:])
                            row0 = s * SUP + mt * 128
                            nc.sync.dma_start(out_d[row0:row0 + 128, :], osb[:])

    nc.compile()
    return nc


def _wrap_idx16(idx):
    """Pack N gather indices into dma_gather's wrapped layout: index j at
    [j % 16, j // 16], replicated across the 8 Q7 cores -> [128, N//16]."""
    n = len(idx)
    arr = np.zeros((16, n // 16), np.int16)
    j = np.arange(n)
    arr[j % 16, j // 16] = idx.astype(np.int16)
    return np.tile(arr, (8, 1))


def _host_prep(inputs):
    rows = np.asarray(inputs["rows"]).astype(np.int64)
    cols = np.asarray(inputs["cols"]).astype(np.int64)
    emat = np.zeros((N_ELEM, BS * BS), dtype=np.float32)
    e_idx = np.arange(N_ELEM)
    emat[e_idx, rows * BS + cols] = 1.0
    emat[e_idx, cols * BS + rows] = -1.0
    oft = np.concatenate([np.asarray(inputs["oft_L"], dtype=np.float32),
                          np.asarray(inputs["oft_R"], dtype=np.float32)], axis=0)
    pout = np.asarray(inputs["perm_out"]).astype(np.int32).reshape(OUT_F, 1)
    inv_pout = np.asarray(inputs["inv_perm_out"]).astype(np.int64)
    inv_pin = np.asarray(inputs["inv_perm_in"]).astype(np.int64)
    gout = np.concatenate([_wrap_idx16(inv_pout[gc * 256:(gc + 1) * 256])
                           for gc in range(NB // 2)], axis=1)
    gin = np.concatenate([_wrap_idx16(inv_pin[kc * 256:(kc + 1) * 256])
                          for kc in range(KB // 2)], axis=1)
    w = np.ascontiguousarray(np.asarray(inputs["W"], dtype=np.float32))
    b = np.asarray(inputs["b"], dtype=np.float32).reshape(OUT_F, 1)
    return emat, oft, pout, gout, gin, w, b


def _in_map(inputs):
    emat, oft, pout, gout, gin, w, b = _host_prep(inputs)
    return {"w": w, "b": b, "oft": oft, "emat": emat,
            "pout": pout, "gout": gout, "gin": gin}


def kernel(**inputs):
    from concourse.bass_utils import run_bass_kernel_spmd

    key = ("full", TOKPC)
    if key not in _CACHE:
        _CACHE[key] = _build(TOKPC)
    nc = _CACHE[key]

    x = np.asarray(inputs["x"], dtype=np.float32).reshape(TOKENS, IN_F)
    base = _in_map(inputs)
    in_maps = []
    for c in range(N_CORES):
        m = dict(base)
        m["xt"] = np.ascontiguousarray(x[c * TOKPC:(c + 1) * TOKPC].T)
        in_maps.append(m)

    res = run_bass_kernel_spmd(nc, in_maps, core_ids=list(range(N_CORES)))
    out = np.concatenate([res.results[c]["out"] for c in range(N_CORES)], axis=0)
    return out.reshape(4, 8192, OUT_F)

